# revision 12
# baseline (speedup 1.0000x reference)
"""Bayesian MLP MC-sample kernel for one TRN2 chip (8 NeuronCores).

Problem: out[s, b, o] for S=32 MC samples of a 3-layer MLP
  dims 256 -> 512 -> 512 -> 64, batch B=2048,
  w_s = z_w[s] * exp(w_log_std) + w_mean   (per-sample reparameterized weights)
  h1 = tanh(x @ w0_s + b0_s); h2 = tanh(h1 @ w1_s + b1_s); out = h2 @ w2_s + b2_s

Sharding: MC-sample axis across the 8 cores (4 samples/core); x and the
mean/log_std parameters are replicated. No cross-core communication.

On-chip layout: everything stays transposed (features on partitions,
batch on the free dim) so the matmul contraction is always the partition
dim and no transposes are needed on device:
  h^T[dout, B] = W^T x^T   via  matmul(psum, lhsT=w[k_chunk, dout_chunk],
                                       rhs=hprev^T[k_chunk, batch_slice])
The host passes x^T in and transposes the (S, 64, B) result back to
(S, B, 64) when gathering -- pure layout work, no FLOPs.

Per-core engine usage:
  PE:  4 samples x 112 matmuls (N=512, f32r -> 1 cycle/row)
  ACT: exp(log_std) once; per-sample tanh(psum + bias) eviction
  DVE: per-sample w = z * sigma + mean (two passes, in place)
  DMA: z shards + replicated params + x^T in, out^T back
"""

import numpy as np

import concourse.bass as bass
import concourse.mybir as mybir
import concourse.tile as tile
from concourse import bacc
from concourse import bass_utils

F32 = mybir.dt.float32
F32R = mybir.dt.float32r
AF = mybir.ActivationFunctionType
ts = bass.ts

S = 32
B = 2048
DIMS = [256, 512, 512, 64]
NCORES = 8
SL = S // NCORES  # samples per core
NSLICE = 512      # moving-dim slice (max for fp32 matmul, = 1 PSUM bank)
NB = B // NSLICE

# knobs test.py may override before the first kernel() call
RUN_KWARGS: dict = {}
LAST_RESULT = None

_CACHE: dict = {}


def _build_nc():
    nc = bacc.Bacc("TRN2", target_bir_lowering=False)

    xT = nc.dram_tensor("xT", [DIMS[0], B], F32, kind="ExternalInput")
    w_mean, w_ls, b_mean, b_ls, z_w, z_b = [], [], [], [], [], []
    for li in range(3):
        din, dout = DIMS[li], DIMS[li + 1]
        w_mean.append(nc.dram_tensor(f"w_mean_{li}", [din, dout], F32, kind="ExternalInput"))
        w_ls.append(nc.dram_tensor(f"w_log_std_{li}", [din, dout], F32, kind="ExternalInput"))
        b_mean.append(nc.dram_tensor(f"b_mean_{li}", [dout], F32, kind="ExternalInput"))
        b_ls.append(nc.dram_tensor(f"b_log_std_{li}", [dout], F32, kind="ExternalInput"))
        z_w.append(nc.dram_tensor(f"z_w_{li}", [SL, din, dout], F32, kind="ExternalInput"))
        z_b.append(nc.dram_tensor(f"z_b_{li}", [SL, dout], F32, kind="ExternalInput"))
    out_d = nc.dram_tensor("out", [SL, DIMS[3], B], F32, kind="ExternalOutput")

    NK = [d // 128 for d in DIMS[:3]]      # k-chunks per layer: 2, 4, 4
    MP = [min(128, d) for d in DIMS[1:]]   # psum partitions:  128, 128, 64
    NM = [d // 128 if d >= 128 else 1 for d in DIMS[1:]]  # m-chunks: 4, 4, 1
    BP = [min(128, d) for d in DIMS[1:]]   # bias partitions
    BC = [max(1, d // 128) for d in DIMS[1:]]  # bias cols

    with tile.TileContext(nc) as tc:
        with (
            tc.tile_pool(name="const", bufs=1) as cpool,
            tc.tile_pool(name="lstmp", bufs=2) as lspool,
            tc.tile_pool(name="z0", bufs=2) as z0p,
            tc.tile_pool(name="z1", bufs=2) as z1p,
            tc.tile_pool(name="z2", bufs=2) as z2p,
            tc.tile_pool(name="w0", bufs=2) as w0p,
            tc.tile_pool(name="w1", bufs=2) as w1p,
            tc.tile_pool(name="w2", bufs=2) as w2p,
            tc.tile_pool(name="b0", bufs=2) as b0p,
            tc.tile_pool(name="b1", bufs=2) as b1p,
            tc.tile_pool(name="b2", bufs=2) as b2p,
            tc.tile_pool(name="h1", bufs=1) as h1p,
            tc.tile_pool(name="h2", bufs=1) as h2p,
            tc.tile_pool(name="osb", bufs=2) as opool,
            tc.tile_pool(name="ps", bufs=2, space="PSUM") as pspool,
        ):
            wpools = [w0p, w1p, w2p]
            zpools = [z0p, z1p, z2p]
            bpools = [b0p, b1p, b2p]

            # ---- constants: x^T, sigma/mean for weights and biases ----
            # one-time staging loads borrow the (still unused) h tile slots
            sigma, mean, sigma_b, mean_b = [], [], [], []
            for li in range(3):
                din, dout = DIMS[li], DIMS[li + 1]
                nk = NK[li]
                ls_t = h2p.tile([128, nk, dout], F32, tag="h1")
                nc.sync.dma_start(ls_t[:], w_ls[li][:].rearrange("(k p) d -> p k d", p=128))
                sg = cpool.tile([128, nk, dout], F32, tag=f"sigma{li}")
                nc.scalar.activation(sg[:], ls_t[:], AF.Exp)
                sigma.append(sg)

                mn = cpool.tile([128, nk, dout], F32, tag=f"mean{li}")
                nc.sync.dma_start(mn[:], w_mean[li][:].rearrange("(k p) d -> p k d", p=128))
                mean.append(mn)

                bp, bc = BP[li], BC[li]
                bls_t = lspool.tile([bp, bc], F32, tag="bls")
                nc.sync.dma_start(bls_t[:], b_ls[li][:].rearrange("(c p) -> p c", p=bp))
                sgb = cpool.tile([bp, bc], F32, tag=f"sigma_b{li}")
                nc.scalar.activation(sgb[:], bls_t[:], AF.Exp)
                sigma_b.append(sgb)

                mnb = cpool.tile([bp, bc], F32, tag=f"mean_b{li}")
                nc.sync.dma_start(mnb[:], b_mean[li][:].rearrange("(c p) -> p c", p=bp))
                mean_b.append(mnb)

            xT_stage = h1p.tile([128, NK[0], B], F32, tag="h0")
            nc.sync.dma_start(xT_stage[:], xT[:].rearrange("(k p) n -> p k n", p=128))
            xT_t = cpool.tile([128, NK[0], B], F32R, tag="xT")
            nc.vector.tensor_copy(xT_t[:], xT_stage[:])

            # per-sample state created lazily by emit_layer
            h_tiles = [dict(), dict()]  # h_tiles[0][s] = h1 of sample s, etc.

            def emit_layer(li, s):
                din, dout = DIMS[li], DIMS[li + 1]
                nk, nm, mp = NK[li], NM[li], MP[li]

                # sampled weights: w = z * sigma + mean
                # (mul in place on the z tile; the add writes the f32r
                # weight tile so matmul sees a rounded-to-f32r producer)
                zt = zpools[li].tile([128, nk, dout], F32, tag=f"z{li}")
                nc.sync.dma_start(
                    zt[:], z_w[li][s].rearrange("(k p) d -> p k d", p=128)
                )
                nc.vector.tensor_mul(zt[:], zt[:], sigma[li][:])
                wt = wpools[li].tile([128, nk, dout], F32R, tag=f"w{li}")
                nc.vector.tensor_add(wt[:], zt[:], mean[li][:])

                # sampled bias
                bp, bc = BP[li], BC[li]
                bt = bpools[li].tile([bp, bc], F32, tag=f"b{li}")
                nc.sync.dma_start(bt[:], z_b[li][s].rearrange("(c p) -> p c", p=bp))
                nc.vector.tensor_mul(bt[:], bt[:], sigma_b[li][:])
                nc.vector.tensor_add(bt[:], bt[:], mean_b[li][:])

                if li == 0:
                    src = xT_t
                else:
                    src = h_tiles[li - 1][s]

                if li < 2:
                    dst = h_tiles[li].get(s)
                    if dst is None:
                        hp = h1p if li == 0 else h2p
                        dst = hp.tile([128, nm, B], F32R, tag=f"h{li}")
                        h_tiles[li][s] = dst

                for m in range(nm):
                    ps = pspool.tile([mp, B], F32, tag="ps")
                    for n in range(NB):
                        for k in range(nk):
                            nc.tensor.matmul(
                                ps[:, ts(n, NSLICE)],
                                wt[:, k, ts(m, mp)],
                                src[:, k, ts(n, NSLICE)],
                                start=(k == 0),
                                stop=(k == nk - 1),
                            )
                    if li < 2:
                        nc.scalar.activation(
                            dst[:, m, :], ps[:], AF.Tanh, bias=bt[:, m : m + 1]
                        )
                    else:
                        osb = opool.tile([mp, B], F32, tag="osb")
                        nc.scalar.activation(
                            osb[:], ps[:], AF.Identity, bias=bt[:, 0:1]
                        )
                        nc.sync.dma_start(out_d[s], osb[:])

            # schedule: delay each sample's last layer until after the next
            # sample's first layer so PE never waits on a tanh eviction
            sched = [(0, 0), (1, 0)]
            for s in range(1, SL):
                sched += [(0, s), (2, s - 1), (1, s)]
            sched.append((2, SL - 1))
            for li, s in sched:
                emit_layer(li, s)
                # h buffers are single-buffered; drop refs once consumed
                if li == 2:
                    h_tiles[0].pop(s, None)
                    h_tiles[1].pop(s, None)

    nc.compile()
    return nc


def _get_nc():
    if "nc" not in _CACHE:
        _CACHE["nc"] = _build_nc()
    return _CACHE["nc"]


def kernel(**inputs) -> np.ndarray:
    global LAST_RESULT
    nc = _get_nc()
    inp = {k: np.asarray(v, dtype=np.float32) for k, v in inputs.items()}

    xT = np.ascontiguousarray(inp["x"].T)
    in_maps = []
    for c in range(NCORES):
        sl = slice(c * SL, (c + 1) * SL)
        m = {"xT": xT}
        for li in range(3):
            m[f"w_mean_{li}"] = inp[f"w_mean_{li}"]
            m[f"w_log_std_{li}"] = inp[f"w_log_std_{li}"]
            m[f"b_mean_{li}"] = inp[f"b_mean_{li}"]
            m[f"b_log_std_{li}"] = inp[f"b_log_std_{li}"]
            m[f"z_w_{li}"] = np.ascontiguousarray(inp[f"z_w_{li}"][sl])
            m[f"z_b_{li}"] = np.ascontiguousarray(inp[f"z_b_{li}"][sl, 0, :])
        in_maps.append(m)

    res = bass_utils.run_bass_kernel_spmd(
        nc, in_maps, core_ids=list(range(NCORES)), **RUN_KWARGS
    )
    LAST_RESULT = res
    full = np.concatenate([res.results[c]["out"] for c in range(NCORES)], axis=0)
    return np.ascontiguousarray(full.transpose(0, 2, 1)).astype(np.float32)


# revision 15
# speedup vs baseline: 1.0409x; 1.0409x over previous
"""Bayesian MLP MC-sample kernel for one TRN2 chip (8 NeuronCores).

Problem: out[s, b, o] for S=32 MC samples of a 3-layer MLP
  dims 256 -> 512 -> 512 -> 64, batch B=2048,
  w_s = z_w[s] * exp(w_log_std) + w_mean   (per-sample reparameterized weights)
  h1 = tanh(x @ w0_s + b0_s); h2 = tanh(h1 @ w1_s + b1_s); out = h2 @ w2_s + b2_s

Sharding: MC-sample axis across the 8 cores (4 samples/core); x and the
mean/log_std parameters are replicated. No cross-core communication.

On-chip layout: everything stays transposed (features on partitions,
batch on the free dim) so the matmul contraction is always the partition
dim and no transposes are needed on device:
  h^T[dout, B] = W^T x^T   via  matmul(psum, lhsT=w[k_chunk, dout_chunk],
                                       rhs=hprev^T[k_chunk, batch_slice])
The host passes x^T in and transposes the (S, 64, B) result back to
(S, B, 64) when gathering -- pure layout work, no FLOPs.

Per-core engine usage:
  PE:  4 samples x 112 matmuls (N=512, f32r -> 1 cycle/row)
  ACT: exp(log_std) once; per-sample tanh(psum + bias) eviction
  DVE: per-sample w = z * sigma + mean (two passes, in place)
  DMA: z shards + replicated params + x^T in, out^T back
"""

import numpy as np

import concourse.bass as bass
import concourse.mybir as mybir
import concourse.tile as tile
from concourse import bacc
from concourse import bass_utils

F32 = mybir.dt.float32
F32R = mybir.dt.float32r
AF = mybir.ActivationFunctionType
ts = bass.ts

S = 32
B = 2048
DIMS = [256, 512, 512, 64]
NCORES = 8
SL = S // NCORES  # samples per core
NSLICE = 512      # moving-dim slice (max for fp32 matmul, = 1 PSUM bank)
NB = B // NSLICE

# knobs test.py may override before the first kernel() call
RUN_KWARGS: dict = {}
LAST_RESULT = None

_CACHE: dict = {}


def _build_nc():
    nc = bacc.Bacc("TRN2", target_bir_lowering=False)

    xT = nc.dram_tensor("xT", [DIMS[0], B], F32, kind="ExternalInput")
    w_mean, w_ls, b_mean, b_ls, z_w, z_b = [], [], [], [], [], []
    for li in range(3):
        din, dout = DIMS[li], DIMS[li + 1]
        w_mean.append(nc.dram_tensor(f"w_mean_{li}", [din, dout], F32, kind="ExternalInput"))
        w_ls.append(nc.dram_tensor(f"w_log_std_{li}", [din, dout], F32, kind="ExternalInput"))
        b_mean.append(nc.dram_tensor(f"b_mean_{li}", [dout], F32, kind="ExternalInput"))
        b_ls.append(nc.dram_tensor(f"b_log_std_{li}", [dout], F32, kind="ExternalInput"))
        z_w.append(nc.dram_tensor(f"z_w_{li}", [SL, din, dout], F32, kind="ExternalInput"))
        z_b.append(nc.dram_tensor(f"z_b_{li}", [SL, dout], F32, kind="ExternalInput"))
    out_d = nc.dram_tensor("out", [SL, DIMS[3], B], F32, kind="ExternalOutput")

    NK = [d // 128 for d in DIMS[:3]]      # k-chunks per layer: 2, 4, 4
    MP = [min(128, d) for d in DIMS[1:]]   # psum partitions:  128, 128, 64
    NM = [d // 128 if d >= 128 else 1 for d in DIMS[1:]]  # m-chunks: 4, 4, 1
    BP = [min(128, d) for d in DIMS[1:]]   # bias partitions
    BC = [max(1, d // 128) for d in DIMS[1:]]  # bias cols

    with tile.TileContext(nc) as tc:
        with (
            tc.tile_pool(name="const", bufs=1) as cpool,
            tc.tile_pool(name="lstmp", bufs=1) as lspool,
            tc.tile_pool(name="z0", bufs=2) as z0p,
            tc.tile_pool(name="z1", bufs=2) as z1p,
            tc.tile_pool(name="z2", bufs=2) as z2p,
            tc.tile_pool(name="w0", bufs=2) as w0p,
            tc.tile_pool(name="w1", bufs=2) as w1p,
            tc.tile_pool(name="w2", bufs=2) as w2p,
            tc.tile_pool(name="b0", bufs=2) as b0p,
            tc.tile_pool(name="b1", bufs=2) as b1p,
            tc.tile_pool(name="b2", bufs=2) as b2p,
            tc.tile_pool(name="h1", bufs=1) as h1p,
            tc.tile_pool(name="h2", bufs=1) as h2p,
            tc.tile_pool(name="osb", bufs=2) as opool,
            tc.tile_pool(name="ps", bufs=2, space="PSUM") as pspool,
        ):
            wpools = [w0p, w1p, w2p]
            zpools = [z0p, z1p, z2p]
            bpools = [b0p, b1p, b2p]

            # constants are emitted lazily, layer by layer, so the first
            # sample's layer-0 matmuls only wait on layer-0 params + x^T
            sigma = [None] * 3
            mean = [None] * 3
            sigma_b = [None] * 3
            mean_b = [None] * 3

            def emit_consts(li):
                din, dout = DIMS[li], DIMS[li + 1]
                nk = NK[li]
                ls_t = lspool.tile([128, nk, dout], F32, tag="ls")
                nc.sync.dma_start(ls_t[:], w_ls[li][:].rearrange("(k p) d -> p k d", p=128))
                sg = cpool.tile([128, nk, dout], F32, tag=f"sigma{li}")
                nc.scalar.activation(sg[:], ls_t[:], AF.Exp)
                sigma[li] = sg

                mn = cpool.tile([128, nk, dout], F32, tag=f"mean{li}")
                nc.sync.dma_start(mn[:], w_mean[li][:].rearrange("(k p) d -> p k d", p=128))
                mean[li] = mn

                bp, bc = BP[li], BC[li]
                bls_t = lspool.tile([bp, bc], F32, tag="bls")
                nc.sync.dma_start(bls_t[:], b_ls[li][:].rearrange("(c p) -> p c", p=bp))
                sgb = cpool.tile([bp, bc], F32, tag=f"sigma_b{li}")
                nc.scalar.activation(sgb[:], bls_t[:], AF.Exp)
                sigma_b[li] = sgb

                mnb = cpool.tile([bp, bc], F32, tag=f"mean_b{li}")
                nc.sync.dma_start(mnb[:], b_mean[li][:].rearrange("(c p) -> p c", p=bp))
                mean_b[li] = mnb

            # per-sample state created lazily by emit_wprep
            h_tiles = [dict(), dict()]  # h_tiles[0][s] = h1 of sample s, etc.
            w_tiles = dict()
            b_tiles = dict()

            def emit_wprep(li, s):
                din, dout = DIMS[li], DIMS[li + 1]
                nk = NK[li]
                # sampled weights: w = z * sigma + mean
                # (mul in place on the z tile; the add writes the f32r
                # weight tile so matmul sees a rounded-to-f32r producer)
                zt = zpools[li].tile([128, nk, dout], F32, tag=f"z{li}")
                nc.sync.dma_start(
                    zt[:], z_w[li][s].rearrange("(k p) d -> p k d", p=128)
                )
                nc.vector.tensor_mul(zt[:], zt[:], sigma[li][:])
                wt = wpools[li].tile([128, nk, dout], F32R, tag=f"w{li}")
                nc.vector.tensor_add(wt[:], zt[:], mean[li][:])
                w_tiles[(li, s)] = wt

                # sampled bias
                bp, bc = BP[li], BC[li]
                bt = bpools[li].tile([bp, bc], F32, tag=f"b{li}")
                nc.sync.dma_start(bt[:], z_b[li][s].rearrange("(c p) -> p c", p=bp))
                nc.vector.tensor_mul(bt[:], bt[:], sigma_b[li][:])
                nc.vector.tensor_add(bt[:], bt[:], mean_b[li][:])
                b_tiles[(li, s)] = bt

            def emit_matmuls(li, s):
                nk, nm, mp = NK[li], NM[li], MP[li]
                wt = w_tiles.pop((li, s))
                bt = b_tiles.pop((li, s))

                if li == 0:
                    src = xT_t
                else:
                    src = h_tiles[li - 1][s]

                if li < 2:
                    hp = h1p if li == 0 else h2p
                    dst = hp.tile([128, nm, B], F32R, tag=f"h{li}")
                    h_tiles[li][s] = dst

                for m in range(nm):
                    ps = pspool.tile([mp, B], F32, tag="ps")
                    for n in range(NB):
                        for k in range(nk):
                            nc.tensor.matmul(
                                ps[:, ts(n, NSLICE)],
                                wt[:, k, ts(m, mp)],
                                src[:, k, ts(n, NSLICE)],
                                start=(k == 0),
                                stop=(k == nk - 1),
                            )
                    if li < 2:
                        nc.scalar.activation(
                            dst[:, m, :], ps[:], AF.Tanh, bias=bt[:, m : m + 1]
                        )
                    else:
                        osb = opool.tile([mp, B], F32, tag="osb")
                        nc.scalar.activation(
                            osb[:], ps[:], AF.Identity, bias=bt[:, 0:1]
                        )
                        nc.sync.dma_start(out_d[s], osb[:])
                        h_tiles[0].pop(s, None)
                        h_tiles[1].pop(s, None)

            # ---- startup: only the layer-0(s=0) critical path up front ----
            emit_consts(0)
            emit_wprep(0, 0)

            # x^T arrives in batch-column slices so matmuls can start before
            # the full 2 MB transfer lands; each slice is cast to f32r
            xT_stage = h1p.tile([128, NK[0], B], F32, tag="h0")
            xT_t = cpool.tile([128, NK[0], B], F32R, tag="xT")
            for n in range(NB):
                nc.sync.dma_start(
                    xT_stage[:, :, ts(n, NSLICE)],
                    xT[:].rearrange("(k p) n -> p k n", p=128)[:, :, ts(n, NSLICE)],
                )
                nc.vector.tensor_copy(
                    xT_t[:, :, ts(n, NSLICE)], xT_stage[:, :, ts(n, NSLICE)]
                )

            emit_matmuls(0, 0)
            emit_consts(1)
            emit_wprep(1, 0)
            emit_matmuls(1, 0)
            emit_consts(2)

            # steady state: delay each sample's last layer until after the
            # next sample's first layer so PE never waits on a tanh eviction
            sched = []
            for s in range(1, SL):
                sched += [(0, s), (2, s - 1), (1, s)]
            sched.append((2, SL - 1))
            for li, s in sched:
                emit_wprep(li, s)
                emit_matmuls(li, s)

    nc.compile()
    return nc


def _get_nc():
    if "nc" not in _CACHE:
        _CACHE["nc"] = _build_nc()
    return _CACHE["nc"]


def kernel(**inputs) -> np.ndarray:
    global LAST_RESULT
    nc = _get_nc()
    inp = {k: np.asarray(v, dtype=np.float32) for k, v in inputs.items()}

    xT = np.ascontiguousarray(inp["x"].T)
    in_maps = []
    for c in range(NCORES):
        sl = slice(c * SL, (c + 1) * SL)
        m = {"xT": xT}
        for li in range(3):
            m[f"w_mean_{li}"] = inp[f"w_mean_{li}"]
            m[f"w_log_std_{li}"] = inp[f"w_log_std_{li}"]
            m[f"b_mean_{li}"] = inp[f"b_mean_{li}"]
            m[f"b_log_std_{li}"] = inp[f"b_log_std_{li}"]
            m[f"z_w_{li}"] = np.ascontiguousarray(inp[f"z_w_{li}"][sl])
            m[f"z_b_{li}"] = np.ascontiguousarray(inp[f"z_b_{li}"][sl, 0, :])
        in_maps.append(m)

    res = bass_utils.run_bass_kernel_spmd(
        nc, in_maps, core_ids=list(range(NCORES)), **RUN_KWARGS
    )
    LAST_RESULT = res
    full = np.concatenate([res.results[c]["out"] for c in range(NCORES)], axis=0)
    return np.ascontiguousarray(full.transpose(0, 2, 1)).astype(np.float32)
